# revision 8
# baseline (speedup 1.0000x reference)
"""Trainium2 Bass kernel for the H3GNN GRU-style GNN cell — fused formulation.

Algebraic restructure vs the reference (per batch element b):
    t_d   = A_d @ hidden                      [S,H]   (d in {in,out})
    gi    = t_in @ V_in^T + t_out @ V_out^T
            + rs'_in x u_in + rs'_out x u_out + c     [S,3H]
    gh    = hidden @ w_hh^T                   [S,3H]
    gates as in the reference
with host-folded weights:
    V_d = Wi_d @ W_d            (Wi1 = w_ih[:, :H], Wi2 = w_ih[:, H:])
    u_d = Wi_d @ b_d
    c   = b_ih + Wi1 @ b_iah + Wi2 @ b_oah + 256*(u_in + u_out)
    rs'_d[i] = sum_j A_d[i,j] - 256           (centered rowsums, host)
This removes the reference's stage-a matmuls (hidden @ W_in/W_out) entirely.

Precision (sim-validated, mean rel err 1.16e-2 vs 2e-2 budget):
    t:    f16 (A, hidden token-major in f16)
    gi_r: fp8 e4m3 DoubleRow (t re-quantized to fp8 on device, V_r x64)
    gi_i, gi_n: f16 (t quantized to f16 on ACT, V x64 in f16)
    gh:   fp8 e4m3 DoubleRow (w_hh x64, hidden e4m3)
    rank-1 rowsum terms: K=2 f16 matmuls into the i/n gate PSUMs (the
    r-gate's term is dropped: double sigmoid damping makes it negligible)
    gates f32; hidden blend + output in f16
All PSUM accumulation is uniformly x64-scaled; the 1/64 rescale folds into
the gate activations' scale parameter.

Schedule notes (all trace-driven):
  - PE mode switches (fp8-DR <-> f16) cost ~300-450ns, so each half's
    matmuls are grouped [all DR | all f16]; DR insts use full 512-free.
  - t8/t16 quantizes run on DVE/ACT (GpSimd tensor ops are ~7us each and
    cannot read PSUM).
  - 10 small warmup matmuls hold the PE p-state through the DMA lead-in;
    too little warmup lets the clock settle at ~2.0GHz for the whole run.
  - last element: chunked epilogue + partition-quarter output DMAs on
    rotating queues to shorten the drain tail.
"""

import os
import sys

import numpy as np

sys.path.insert(0, "/opt/trn_rl_repo")

import ml_dtypes  # noqa: E402

from concourse import bacc, mybir, tile  # noqa: E402
from concourse.bass_utils import run_bass_kernel_spmd  # noqa: E402

B, S, H = 128, 512, 256
N_CORES = 8
M_PER_CORE = B // N_CORES  # 16

f32 = mybir.dt.float32
f32r = mybir.dt.float32r
f16 = mybir.dt.float16
f8e4 = mybir.dt.float8e4
u8 = mybir.dt.uint8

AF = mybir.ActivationFunctionType
ALU = mybir.AluOpType
PM = mybir.MatmulPerfMode

E4M3 = ml_dtypes.float8_e4m3
F16 = np.float16

WS = 64.0       # uniform PSUM scale (fp8 + f16 weights all x64)
N_WARM = 10

LAST_RESULT = None


def _build(n_batch=M_PER_CORE):
    nc = bacc.Bacc("TRN2", target_bir_lowering=False, debug=False,
                   num_devices=N_CORES)

    at_d = nc.dram_tensor("at16", [n_batch, 128, 4096], f16, kind="ExternalInput").ap()
    hn_d = nc.dram_tensor("hn16", [n_batch, 128, 1024], f16, kind="ExternalInput").ap()
    ht_d = nc.dram_tensor("ht16", [n_batch, 128, 1024], f16, kind="ExternalInput").ap()
    ht8_d = nc.dram_tensor("ht8", [n_batch, 128, 2, 512], u8, kind="ExternalInput").ap()
    rs_d = nc.dram_tensor("rs2", [n_batch, 2, 512], f16, kind="ExternalInput").ap()
    v16_d = nc.dram_tensor("v16", [128, 1024], f16, kind="ExternalInput").ap()
    vi16_d = nc.dram_tensor("vi16", [128, 1024], f16, kind="ExternalInput").ap()
    vr8_d = nc.dram_tensor("vr8", [128, 2, 512], u8, kind="ExternalInput").ap()
    whh8_d = nc.dram_tensor("whh8", [128, 2, 768], u8, kind="ExternalInput").ap()
    u2_d = nc.dram_tensor("u2", [2, 768], f16, kind="ExternalInput").ap()
    bri_d = nc.dram_tensor("b_ri", [128, 4], f32, kind="ExternalInput").ap()
    bhn_d = nc.dram_tensor("b_hn", [128, 2], f32, kind="ExternalInput").ap()
    bn2_d = nc.dram_tensor("b_n2", [128, 2], f32, kind="ExternalInput").ap()
    out_d = nc.dram_tensor("outt", [n_batch, 128, 1024], f16, kind="ExternalOutput").ap()

    with tile.TileContext(nc) as tc:
        with (
            tc.tile_pool(name="wpool", bufs=1) as wpool,
            tc.tile_pool(name="apool", bufs=3) as apool,
            tc.tile_pool(name="hpool", bufs=3) as hpool,
            tc.tile_pool(name="qpool", bufs=3) as qpool,
            tc.tile_pool(name="gates", bufs=3) as gpool,
            tc.tile_pool(name="ps_t", bufs=1, space="PSUM") as ps_t,
            tc.tile_pool(name="ps_g", bufs=1, space="PSUM") as ps_g,
        ):
            # --- replicated weights / biases ---
            v16_sb = wpool.tile([128, 1024], f16)
            vi16_sb = wpool.tile([128, 1024], f16)
            vr8_sb = wpool.tile([128, 2, 512], f8e4)
            whh8_sb = wpool.tile([128, 2, 768], f8e4)
            u2_sb = wpool.tile([2, 768], f16)
            bri_sb = wpool.tile([128, 4], f32)
            bhn_sb = wpool.tile([128, 2], f32)
            bn2_sb = wpool.tile([128, 2], f32)
            warm = wpool.tile([128, 512], f32)

            # PE warmup: keep PE busy through DMA startup + p-state ramp.
            # Small matmuls: if the ramp advances per instruction this exits
            # the slow p-states in far less wall time than 512-free warmups.
            nc.vector.memset(warm[:], 0.0)
            for _ in range(N_WARM):
                pw = ps_t.tile([128, 512], f32, tag="t00")
                nc.tensor.matmul(pw[:, 0:64], warm[:, 0:128].bitcast(f32r),
                                 warm[:, 0:64].bitcast(f32r), start=True, stop=True)

            def load_elem(m, split_at=False):
                hn_sb = hpool.tile([128, 1024], f16, tag="hn")
                nc.sync.dma_start(hn_sb[:], hn_d[m])
                at_sb = apool.tile([128, 4096], f16, tag="at")
                if split_at:
                    for jc in range(8):
                        nc.sync.dma_start(at_sb[:, jc * 512:(jc + 1) * 512],
                                          at_d[m][:, jc * 512:(jc + 1) * 512])
                else:
                    nc.sync.dma_start(at_sb[:], at_d[m])
                ht_sb = hpool.tile([128, 1024], f16, tag="ht")
                nc.sync.dma_start(ht_sb[:], ht_d[m])
                ht8_sb = hpool.tile([128, 2, 512], f8e4, tag="ht8")
                nc.sync.dma_start(ht8_sb[:], ht8_d[m].bitcast(f8e4))
                rs_sb = hpool.tile([2, 512], f16, tag="rs")
                nc.sync.dma_start(rs_sb[:], rs_d[m])
                return (at_sb, hn_sb, ht_sb, ht8_sb, rs_sb)

            def stage_t(data):
                """t^T[d][gc] = sum_jc hn16[:,jc,gc*128:+128]^T @ at16[:,jc,d,:].
                jc-major with d inner so each lhsT load serves two matmuls."""
                at_sb, hn_sb = data[0], data[1]
                pts = {}
                for gc in range(2):
                    for d in range(2):
                        pts[d * 2 + gc] = ps_t.tile([128, 512], f32, tag=f"t{d}{gc}", name=f"pt{d}{gc}")
                    for jc in range(4):
                        for d in range(2):
                            nc.tensor.matmul(
                                pts[d * 2 + gc][:],
                                hn_sb[:, jc * 256 + gc * 128: jc * 256 + (gc + 1) * 128],
                                at_sb[:, jc * 1024 + d * 512: jc * 1024 + (d + 1) * 512],
                                start=(jc == 0), stop=(jc == 3),
                                skip_group_check=True,
                            )
                return [pts[k] for k in range(4)]

            def quantize(pts):
                """t8 via DVE (fp8 writes run 2x on DVE), t16 via ACT —
                both straight from PSUM. GpSimd is far too slow for this."""
                t8 = []
                for d in range(2):
                    t8_d = qpool.tile([128, 2, 512], f8e4, tag=f"t8{d}")
                    for gc in range(2):
                        nc.vector.tensor_scalar_mul(t8_d[:, gc, :], pts[d * 2 + gc][:], 1.0)
                    t8.append(t8_d)
                t16 = []
                for d in range(2):
                    for gc in range(2):
                        tt = qpool.tile([128, 512], f16, tag=f"t16{d}{gc}")
                        nc.scalar.copy(tt[:], pts[d * 2 + gc][:])
                        t16.append(tt)
                return t8, t16

            def gates(m, data, t8, t16, last=False):
                _, _, ht_sb, ht8_sb, rs_sb = data
                rs_r = rs_sb[:]
                out_sb = gpool.tile([128, 1024], f16, tag="out")
                for c in range(2):
                    # --- DR (fp8) block. Bank order follows the previous
                    # half's epilogue read order (r_g first, i_g second,
                    # v last) so bank WARs don't stall the PE. ---
                    p_r = ps_g.tile([128, 512], f32, tag="pr")
                    for d in range(2):
                        nc.tensor.matmul(
                            p_r[:],
                            vr8_sb[:, :, d * 256 + c * 128: d * 256 + (c + 1) * 128],
                            t8[d][:, :, :],
                            start=(d == 0), stop=False,
                            perf_mode=PM.DoubleRow, skip_group_check=True,
                        )
                    nc.tensor.matmul(
                        p_r[:],
                        whh8_sb[:, :, c * 128:(c + 1) * 128],
                        ht8_sb[:, :, :],
                        start=False, stop=True,
                        perf_mode=PM.DoubleRow, skip_group_check=True,
                    )
                    p_i = ps_g.tile([128, 512], f32, tag="pi")
                    nc.tensor.matmul(
                        p_i[:],
                        whh8_sb[:, :, (2 + c) * 128:(3 + c) * 128],
                        ht8_sb[:, :, :],
                        start=True, stop=False,
                        perf_mode=PM.DoubleRow, skip_group_check=True,
                    )
                    ph_n = ps_g.tile([128, 512], f32, tag="phn")
                    nc.tensor.matmul(
                        ph_n[:],
                        whh8_sb[:, :, (4 + c) * 128:(5 + c) * 128],
                        ht8_sb[:, :, :],
                        start=True, stop=True,
                        perf_mode=PM.DoubleRow, skip_group_check=True,
                    )
                    # --- f16 block ---
                    for k in range(4):
                        nc.tensor.matmul(
                            p_i[:],
                            vi16_sb[:, k * 256 + c * 128: k * 256 + (c + 1) * 128],
                            t16[k][:],
                            start=False, stop=False, skip_group_check=True,
                        )
                    nc.tensor.matmul(p_i[:], u2_sb[:, 256 + c * 128: 256 + (c + 1) * 128],
                                     rs_r, start=False, stop=True,
                                     skip_group_check=True)
                    pg_n = ps_g.tile([128, 512], f32, tag="pgn")
                    for k in range(4):
                        nc.tensor.matmul(
                            pg_n[:],
                            v16_sb[:, k * 256 + c * 128: k * 256 + (c + 1) * 128],
                            t16[k][:],
                            start=(k == 0), stop=False, skip_group_check=True,
                        )
                    nc.tensor.matmul(pg_n[:], u2_sb[:, 512 + c * 128: 512 + (c + 1) * 128],
                                     rs_r, start=False, stop=True,
                                     skip_group_check=True)

                    # ---- epilogue (chunked on the last element to cut
                    # the serial drain + spread its out-DMA over engines) ----
                    hseg = ht_sb[:, c * 512:(c + 1) * 512]
                    chunks = ((0, 256), (256, 512)) if last else ((0, 512),)
                    r_g = gpool.tile([128, 512], f32, tag="r_g")
                    i_g = gpool.tile([128, 512], f32, tag="i_g")
                    v = gpool.tile([128, 512], f32, tag="v")
                    w = gpool.tile([128, 512], f32, tag="w")
                    n_g = gpool.tile([128, 512], f32, tag="n_g")
                    dd = gpool.tile([128, 512], f32, tag="dd")
                    ee = gpool.tile([128, 512], f32, tag="ee")
                    for s0, s1 in chunks:
                        sl = slice(s0, s1)
                        nc.scalar.activation(r_g[:, sl], p_r[:, sl], AF.Sigmoid,
                                             bias=bri_sb[:, c:c + 1], scale=1.0 / WS)
                        nc.scalar.activation(i_g[:, sl], p_i[:, sl], AF.Sigmoid,
                                             bias=bri_sb[:, 2 + c:3 + c], scale=1.0 / WS)
                        nc.vector.scalar_tensor_tensor(v[:, sl], ph_n[:, sl],
                                                       bhn_sb[:, c:c + 1],
                                                       r_g[:, sl], ALU.add, ALU.mult)
                        nc.vector.tensor_tensor(w[:, sl], pg_n[:, sl], v[:, sl], ALU.add)
                        nc.scalar.activation(n_g[:, sl], w[:, sl], AF.Tanh,
                                             bias=bn2_sb[:, c:c + 1], scale=1.0 / WS)
                        nc.vector.tensor_tensor(dd[:, sl], n_g[:, sl], hseg[:, sl],
                                                ALU.subtract)
                        nc.vector.tensor_tensor(ee[:, sl], i_g[:, sl], dd[:, sl],
                                                ALU.mult)
                        nc.vector.tensor_tensor(out_sb[:, c * 512 + s0: c * 512 + s1],
                                                hseg[:, sl], ee[:, sl], ALU.add)
                        if last and s1 == 512:
                            # full c-half computed: DMA as partition quarters
                            # (full-width rows -> fast) on rotating engines
                            engs = [nc.gpsimd, nc.sync, nc.scalar, nc.sync]
                            for qi in range(4):
                                p0, p1 = qi * 32, (qi + 1) * 32
                                engs[qi].dma_start(
                                    out_d[m][p0:p1, c * 512:(c + 1) * 512],
                                    out_sb[p0:p1, c * 512:(c + 1) * 512])
                        elif not last:
                            nc.gpsimd.dma_start(
                                out_d[m][:, c * 512 + s0: c * 512 + s1],
                                out_sb[:, c * 512 + s0: c * 512 + s1])

            # ---- prologue DMAs ----
            data0 = load_elem(0, split_at=True)
            nc.sync.dma_start(vr8_sb[:], vr8_d.bitcast(f8e4))
            nc.sync.dma_start(whh8_sb[:], whh8_d.bitcast(f8e4))
            nc.sync.dma_start(vi16_sb[:], vi16_d)
            nc.sync.dma_start(v16_sb[:], v16_d)
            nc.sync.dma_start(u2_sb[:], u2_d)
            nc.sync.dma_start(bri_sb[:], bri_d)
            nc.sync.dma_start(bhn_sb[:], bhn_d)
            nc.sync.dma_start(bn2_sb[:], bn2_d)

            # ---- software-pipelined main loop ----
            data = {0: data0}
            pts = stage_t(data[0])
            tq = {0: quantize(pts)}
            for m in range(n_batch):
                if m + 1 < n_batch:
                    data[m + 1] = load_elem(m + 1)
                    pts = stage_t(data[m + 1])
                    tq[m + 1] = quantize(pts)
                t8, t16 = tq.pop(m)
                gates(m, data.pop(m), t8, t16, last=(m == n_batch - 1))

    nc.compile()
    return nc


def _host_pack(A, hidden, W_in, b_in, W_out, b_out, b_iah, b_oah,
               w_ih, b_ih, w_hh, b_hh):
    """Host-side layout transforms + weight folding."""
    A = np.asarray(A, dtype=np.float32)
    hidden = np.asarray(hidden, dtype=np.float32)
    W_in = np.asarray(W_in, np.float32)
    W_out = np.asarray(W_out, np.float32)
    w_ih = np.asarray(w_ih, np.float32)
    w_hh = np.asarray(w_hh, np.float32)
    b_in = np.asarray(b_in, np.float32)
    b_out = np.asarray(b_out, np.float32)
    b_iah = np.asarray(b_iah, np.float32)
    b_oah = np.asarray(b_oah, np.float32)
    b_ih = np.asarray(b_ih, np.float32)
    b_hh = np.asarray(b_hh, np.float32)

    # per-element tensors
    # at16[b, p, jc, d, i] = A[b, i, d*512 + jc*128 + p]
    at16 = np.ascontiguousarray(
        A.reshape(B, S, 2, 4, 128).transpose(0, 4, 3, 2, 1).astype(F16)
    ).reshape(B, 128, 4096)
    # hn16[b, p, jc, g] = hidden[b, jc*128+p, g]
    hn16 = np.ascontiguousarray(
        hidden.reshape(B, 4, 128, H).transpose(0, 2, 1, 3).astype(F16)
    ).reshape(B, 128, 1024)
    # ht[b, p, hc, s] = hidden[b, s, hc*128+p]
    ht_perm = hidden.reshape(B, S, 2, 128).transpose(0, 3, 2, 1)  # [B,128,2,S]
    ht16 = np.ascontiguousarray(ht_perm.astype(F16)).reshape(B, 128, 1024)
    ht8 = np.ascontiguousarray(ht_perm.astype(E4M3).view(np.uint8))  # [B,128,2,512]
    # rs2[b, d, i] = sum_j A[b, i, d*512+j] - 256
    rs2 = np.ascontiguousarray(
        (A.reshape(B, S, 2, S).sum(axis=3).transpose(0, 2, 1) - 256.0).astype(F16)
    )  # [B, 2, 512]

    # folded weights
    Wi1, Wi2 = w_ih[:, :H], w_ih[:, H:]
    V = np.stack([Wi1 @ W_in, Wi2 @ W_out])       # [2(d), 3H, H]
    u_d = np.stack([Wi1 @ b_in, Wi2 @ b_out])     # [2(d), 3H]
    cc = b_ih + Wi1 @ b_iah + Wi2 @ b_oah + 256.0 * (u_d[0] + u_d[1])

    # v16[p, (d*2+gc)*256 + c*128 + r] = WS * V[d, 512 + c*128 + r, gc*128 + p]
    def pack_f16_third(t):
        # rows t*256 .. t*256+255 of V
        blk = V[:, t * 256:(t + 1) * 256, :] * WS    # [2, 256, 256]
        blk = blk.reshape(2, 2, 128, 2, 128)         # [d, c, r, gc, p]
        return np.ascontiguousarray(
            blk.transpose(4, 0, 3, 1, 2).astype(F16)).reshape(128, 1024)
        # order: [p, d, gc, c, r]

    v16 = pack_f16_third(2)   # n-third
    vi16 = pack_f16_third(1)  # i-third
    # vr8[p, u, d*256 + c*128 + r] = e4m3(WS * V[d, c*128 + r, u*128 + p])
    vr8 = np.ascontiguousarray(
        (V[:, 0:256, :] * WS).reshape(2, 2, 128, 2, 128)  # [d, c, r, u, p]
        .transpose(4, 3, 0, 1, 2).astype(E4M3).view(np.uint8)).reshape(128, 2, 512)
    # whh8[p, u, rc*128 + r] = e4m3(WS * w_hh[rc*128+r, u*128+p])
    whh8 = np.ascontiguousarray(
        (w_hh * WS).reshape(6, 128, 2, 128)  # [rc, r, u, p]
        .transpose(3, 2, 0, 1).astype(E4M3).view(np.uint8)).reshape(128, 2, 768)
    # u2[d, rr] = WS * u_d[rr]  (uniform x64; tanh/sigmoid rescale by 1/64)
    u2 = np.ascontiguousarray((u_d * WS).astype(F16))
    # biases
    bb = cc + b_hh
    bri = np.stack([bb[0:128], bb[128:256], bb[256:384], bb[384:512]], axis=1)
    bhn = np.stack([b_hh[512:640] * WS, b_hh[640:768] * WS], axis=1)
    bn2 = np.stack([cc[512:640], cc[640:768]], axis=1)

    shared = {
        "v16": v16, "vi16": vi16, "vr8": vr8, "whh8": whh8,
        "u2": u2,
        "b_ri": np.ascontiguousarray(bri, np.float32),
        "b_hn": np.ascontiguousarray(bhn, np.float32),
        "b_n2": np.ascontiguousarray(bn2, np.float32),
    }
    return at16, hn16, ht16, ht8, rs2, shared


def kernel(A, hidden, mask, W_in, b_in, W_out, b_out, b_iah, b_oah,
           w_ih, b_ih, w_hh, b_hh, **_unused):
    global LAST_RESULT
    at16, hn16, ht16, ht8, rs2, shared = _host_pack(
        A, hidden, W_in, b_in, W_out, b_out, b_iah, b_oah,
        w_ih, b_ih, w_hh, b_hh)
    nc = _build()
    in_maps = []
    for core in range(N_CORES):
        sl = slice(core * M_PER_CORE, (core + 1) * M_PER_CORE)
        in_maps.append({"at16": at16[sl], "hn16": hn16[sl], "ht16": ht16[sl],
                        "ht8": ht8[sl], "rs2": rs2[sl], **shared})
    trace = bool(os.environ.get("KERNEL_TRACE"))
    if trace:
        try:
            import prof_shim
            prof_shim.install()
        except Exception:
            trace = False
    res = run_bass_kernel_spmd(nc, in_maps, list(range(N_CORES)), trace=trace)
    LAST_RESULT = res
    outt = np.concatenate([res.results[c]["outt"] for c in range(N_CORES)], axis=0)
    # invert: out[b, s, hc*128+p] = outt[b, p, hc*512 + s]
    out = np.ascontiguousarray(
        outt.astype(np.float32).reshape(B, 128, 2, S).transpose(0, 3, 2, 1)
    ).reshape(B, S, H)
    return out
